# revision 10
# baseline (speedup 1.0000x reference)
"""Log2Quantizer Trainium2 kernel (raw Bass, no Tile).

Math: the reference's sort/std/rank machinery is dead code (bit_token is
unconditionally overwritten with n_bits), so the computation reduces to:
    delta[b,t] = max over (h,c) of x[b,h,t,c]
    out = delta * 2^(round(log2(max(x/delta, 1e-8))))
i.e. snap x/delta to the nearest power of two in log space, rescale by delta.

Bit-trick (no transcendentals): round(log2 r) = floor(log2(r/sqrt2)) + 1:
    q   = x * (isqrt2/delta)                 per-token scale (ACT, M1)
    p2  = bitcast_f32(bits(q) & 0x7F800000)  2^floor(log2 q)   (DVE, AND)
    out = p2 * (2*delta)                     exact fp32 mult   (M2, split)
x==0 gives q=0 -> p2=+0.0 -> out=0 (reference's clamp yields delta*2^-27
~ 7e-9 there; abs err 7e-9 on the rare exact-zero input).

Sharding: data-parallel over batch dim b (8 rows -> 8 cores), no comms.
Layout: t split into TC=512-token chunks; partition dim = t-block of tt=4 so
each partition line is one contiguous 1KB run per h in DRAM. 1KB descriptors
already saturate the per-DMA-engine bus (22.5 B/ns * 16 engines ~ 360 GB/s
aggregate, the real bottleneck at 25.2 MB total traffic -> ~70us floor).

Engine pipeline (vs the all-DVE baseline at 98.6us whose DVE ran 86us).
The per-token scalar ops must be sliced into 4 per-q [128,x] ops whose
scalar is a [128,1] AP; measured slice costs: DVE 613ns, ACT 1013ns.
  Sync:   load DMAs (own HWDGE ring). NBUF = n_chunks = whole tensor
          double-buffered in SBUF (2*8*12KB/partition), loads never wait.
  ACT:    M1 all 4 slices + M2 slices 2..3 (activation-Copy, scale AP).
  DVE:    2-stage max-reduce (contiguous-X over c, then 48-elem reduce
          over h), reciprocal, 2 tiny per-token ops, flat single-op AND,
          M2 slices 0..1 (tensor_scalar_mul, scalar AP).
  GpSimd: store DMA issue only (descriptor writes, no SBUF data traffic).
Per chunk: DVE ~7.5us, ACT ~6.3us -> ~60us each, under the ~70us DMA
floor, and each engine runs a straight lag-1 pipeline (ACT works on chunk
ci's M1 while DVE does ci-1's AND; no blocking zigzag).

Buffers ping-pong, no in-place ops, NBUF = n_chunks so no reuse at all:
M1 xt->wt, AND wt->xt, M2 xt->wt, store from wt.

Sems -- every data handoff (same-engine included: engines pipeline, e.g.
an issued DMA can read an earlier op's output before its data lands)
waits on the producer's counting increment:
  load_sem:  +16 per load DMA; DVE waits 16*(ci+1) (one FIFO ring, in-order)
  scal_sem:  +2 per chunk by DVE (inv', d2); ACT M1 waits 2*(ci+1)
  m1_sem:    +1 per ACT M1 slice; DVE AND waits tt*(ci+1)
  and_sem:   +1 per chunk by DVE AND; ACT M2b waits ci+1
  m2a_sem:   +1 per DVE M2 slice;  gp store waits 2*(ci+1)
  m2b_sem:   +1 per ACT M2 slice;  gp store waits 2*(ci+1)
  dve_sem:   DVE-internal RAW fences (reduce1->reduce2->recip->tinies)
  store_sem: +16 per store DMA; sync tail-waits 16*n_chunks (output flushed)
"""

from contextlib import ExitStack

import numpy as np

import concourse.bass as bass
import concourse.mybir as mybir
from concourse.bass_utils import run_bass_kernel_spmd

B, H, T, C = 8, 12, 4096, 64
N_CORES = 8
P = 128          # SBUF partitions
TC = 512         # tokens per chunk (pipeline granularity)

ISQRT2 = 0.7071067811865476
EXP_MASK = 0x7F800000
M2_DVE = 2       # M2 slices 0..M2_DVE-1 on DVE, the rest on ACT

_nc_cache = {}


def _build_nc():
    if "nc" in _nc_cache:
        return _nc_cache["nc"]
    f32 = mybir.dt.float32
    i32 = mybir.dt.int32
    OP = mybir.AluOpType
    AF = mybir.ActivationFunctionType

    nc = bass.Bass()
    x_in = nc.declare_dram_parameter("x", [H, T, C], f32, isOutput=False)
    y_out = nc.declare_dram_parameter("y", [H, T, C], f32, isOutput=True)

    n_chunks = T // TC
    tt = TC // P
    FREE = H * tt * C
    NBUF = n_chunks

    def src_ap(ci):
        return x_in[:, ci * TC : (ci + 1) * TC, :].rearrange(
            "h (p q) c -> p h (q c)", p=P
        )

    def dst_ap(ci):
        return y_out[:, ci * TC : (ci + 1) * TC, :].rearrange(
            "h (p q) c -> p h (q c)", p=P
        )

    with ExitStack() as ctx:
        xt = [
            ctx.enter_context(nc.sbuf_tensor(f"xt{j}", [P, FREE], f32))
            for j in range(NBUF)
        ]
        wt = [
            ctx.enter_context(nc.sbuf_tensor(f"wt{j}", [P, FREE], f32))
            for j in range(NBUF)
        ]
        red = ctx.enter_context(nc.sbuf_tensor("red", [P, H * tt], f32))
        delta = [
            ctx.enter_context(nc.sbuf_tensor(f"delta{j}", [P, tt], f32))
            for j in range(NBUF)
        ]
        inv = [
            ctx.enter_context(nc.sbuf_tensor(f"inv{j}", [P, tt], f32))
            for j in range(NBUF)
        ]
        d2 = [
            ctx.enter_context(nc.sbuf_tensor(f"d2_{j}", [P, tt], f32))
            for j in range(NBUF)
        ]

        load_sem = ctx.enter_context(nc.semaphore("load_sem"))
        store_sem = ctx.enter_context(nc.semaphore("store_sem"))
        scal_sem = ctx.enter_context(nc.semaphore("scal_sem"))
        m1_sem = ctx.enter_context(nc.semaphore("m1_sem"))
        and_sem = ctx.enter_context(nc.semaphore("and_sem"))
        m2a_sem = ctx.enter_context(nc.semaphore("m2a_sem"))
        m2b_sem = ctx.enter_context(nc.semaphore("m2b_sem"))
        dve_sem = ctx.enter_context(nc.semaphore("dve_sem"))

        block = ctx.enter_context(nc.Block())

        def view4(t):
            return t[:].rearrange("p (h q c) -> p h q c", h=H, c=C)

        @block.sync
        def _(sync):
            # loads only; SP HWDGE ring; NBUF == n_chunks so no buffer reuse
            for ci in range(n_chunks):
                sync.dma_start(out=xt[ci][:], in_=src_ap(ci)).then_inc(
                    load_sem, 16
                )
            # output-flush guarantee before NEFF end
            sync.wait_ge(store_sem, 16 * n_chunks)

        @block.vector
        def _(vector):
            b = 0
            for ci in range(n_chunks):
                xt3 = xt[ci][:].rearrange("p (hq c) -> p hq c", c=C)
                vector.wait_ge(load_sem, 16 * (ci + 1))
                # delta = max over (h, c) in two stages: contiguous X over c,
                # then the tiny 48-element strided reduce over h
                vector.reduce_max(
                    out=red[:], in_=xt3, axis=mybir.AxisListType.X
                ).then_inc(dve_sem, 1)
                vector.wait_ge(dve_sem, b + 1)
                vector.reduce_max(
                    out=delta[ci][:],
                    in_=red[:].rearrange("p (h q) -> p q h", h=H),
                    axis=mybir.AxisListType.X,
                ).then_inc(dve_sem, 1)
                vector.wait_ge(dve_sem, b + 2)
                vector.reciprocal(inv[ci][:], delta[ci][:]).then_inc(dve_sem, 1)
                vector.wait_ge(dve_sem, b + 3)
                # inv' = isqrt2/delta (M1 scale); d2 = 2*delta (M2 scale)
                vector.tensor_scalar_mul(inv[ci][:], inv[ci][:], ISQRT2).then_inc(
                    scal_sem, 1
                )
                vector.tensor_scalar_mul(d2[ci][:], delta[ci][:], 2.0).then_inc(
                    scal_sem, 1
                )
                b += 3
                if ci >= 1:
                    k = ci - 1
                    # AND: p2 = bits(q) & mask, wt -> xt, one flat op
                    vector.wait_ge(m1_sem, tt * ci)
                    vector.tensor_scalar(
                        out=xt[k][:].bitcast(i32),
                        in0=wt[k][:].bitcast(i32),
                        scalar1=EXP_MASK,
                        scalar2=None,
                        op0=OP.bitwise_and,
                    ).then_inc(and_sem, 1)
                    # M2 slices 0..M2_DVE-1: out = p2 * d2, xt -> wt (the
                    # AND above is this stream's own op: in-order + the
                    # consumer-side fence below)
                    vector.wait_ge(and_sem, ci)
                    for s in range(M2_DVE):
                        vector.tensor_scalar_mul(
                            view4(wt[k])[:, :, s, :],
                            view4(xt[k])[:, :, s, :],
                            d2[k][:, s : s + 1],
                        ).then_inc(m2a_sem, 1)
            k = n_chunks - 1
            vector.wait_ge(m1_sem, tt * n_chunks)
            vector.tensor_scalar(
                out=xt[k][:].bitcast(i32),
                in0=wt[k][:].bitcast(i32),
                scalar1=EXP_MASK,
                scalar2=None,
                op0=OP.bitwise_and,
            ).then_inc(and_sem, 1)
            vector.wait_ge(and_sem, n_chunks)
            for s in range(M2_DVE):
                vector.tensor_scalar_mul(
                    view4(wt[k])[:, :, s, :],
                    view4(xt[k])[:, :, s, :],
                    d2[k][:, s : s + 1],
                ).then_inc(m2a_sem, 1)

        @block.scalar
        def _(scalar):
            # M1 (all slices) + M2 slices M2_DVE..tt-1; activation-Copy with
            # per-partition scale AP
            for ci in range(n_chunks):
                scalar.wait_ge(scal_sem, 2 * (ci + 1))
                for s in range(tt):
                    scalar.activation(
                        out=view4(wt[ci])[:, :, s, :],
                        in_=view4(xt[ci])[:, :, s, :],
                        func=AF.Copy,
                        scale=inv[ci][:, s : s + 1],
                    ).then_inc(m1_sem, 1)
                if ci >= 1:
                    k = ci - 1
                    scalar.wait_ge(and_sem, ci)
                    for s in range(M2_DVE, tt):
                        scalar.activation(
                            out=view4(wt[k])[:, :, s, :],
                            in_=view4(xt[k])[:, :, s, :],
                            func=AF.Copy,
                            scale=d2[k][:, s : s + 1],
                        ).then_inc(m2b_sem, 1)
            k = n_chunks - 1
            scalar.wait_ge(and_sem, n_chunks)
            for s in range(M2_DVE, tt):
                scalar.activation(
                    out=view4(wt[k])[:, :, s, :],
                    in_=view4(xt[k])[:, :, s, :],
                    func=AF.Copy,
                    scale=d2[k][:, s : s + 1],
                ).then_inc(m2b_sem, 1)

        @block.gpsimd
        def _(gpsimd):
            # store issue only; gp HWDGE ring (descriptor writes, no SBUF
            # data ops -> no DVE port contention)
            for ci in range(n_chunks):
                gpsimd.wait_ge(m2a_sem, M2_DVE * (ci + 1))
                gpsimd.wait_ge(m2b_sem, (tt - M2_DVE) * (ci + 1))
                gpsimd.dma_start(out=dst_ap(ci), in_=wt[ci][:]).then_inc(
                    store_sem, 16
                )

    _nc_cache["nc"] = nc
    return nc


def kernel(x: np.ndarray) -> np.ndarray:
    assert x.shape == (B, H, T, C) and x.dtype == np.float32
    nc = _build_nc()
    in_maps = [{"x": np.ascontiguousarray(x[i])} for i in range(N_CORES)]
    res = run_bass_kernel_spmd(nc, in_maps, list(range(N_CORES)))
    out = np.stack([res.results[i]["y"] for i in range(N_CORES)], axis=0)
    return out


# revision 12
# speedup vs baseline: 1.2413x; 1.2413x over previous
"""Log2Quantizer Trainium2 kernel (raw Bass, no Tile).

Math: the reference's sort/std/rank machinery is dead code (bit_token is
unconditionally overwritten with n_bits), so the computation reduces to:
    delta[b,t] = max over (h,c) of x[b,h,t,c]
    out = delta * 2^(round(log2(max(x/delta, 1e-8))))
i.e. snap x/delta to the nearest power of two in log space, rescale by delta.

Bit-trick (no transcendentals): round(log2 r) = floor(log2(r/sqrt2)) + 1:
    q   = x * (isqrt2/delta)                 per-token scale (ACT, M1)
    p2  = bitcast_f32(bits(q) & 0x7F800000)  2^floor(log2 q)   (DVE, AND)
    out = p2 * (2*delta)                     exact fp32 mult   (M2, split)
x==0 gives q=0 -> p2=+0.0 -> out=0 (reference's clamp yields delta*2^-27
~ 7e-9 there; abs err 7e-9 on the rare exact-zero input).

Sharding: data-parallel over batch dim b (8 rows -> 8 cores), no comms.
Layout: t split into TC=512-token chunks; partition dim = t-block of tt=4 so
each partition line is one contiguous 1KB run per h in DRAM. 1KB descriptors
already saturate the per-DMA-engine bus (22.5 B/ns * 16 engines ~ 360 GB/s
aggregate, the real bottleneck at 25.2 MB total traffic -> ~70us floor).

Engine pipeline (vs the all-DVE baseline at 98.6us whose DVE ran 86us).
The per-token scalar ops must be sliced into 4 per-q [128,x] ops whose
scalar is a [128,1] AP; measured slice costs: DVE 613ns, ACT 1013ns.
  Sync:   load DMAs (own HWDGE ring). NBUF = n_chunks = whole tensor
          double-buffered in SBUF (2*8*12KB/partition), loads never wait.
  ACT:    M1 all 4 slices + M2 slices 2..3 (activation-Copy, scale AP).
  DVE:    2-stage max-reduce (contiguous-X over c, then 48-elem reduce
          over h), reciprocal, 2 tiny per-token ops, flat single-op AND,
          M2 slices 0..1 (tensor_scalar_mul, scalar AP).
  GpSimd: store DMA issue only (descriptor writes, no SBUF data traffic).
Per chunk: DVE ~7.5us, ACT ~6.3us -> ~60us each, under the ~70us DMA
floor, and each engine runs a straight lag-1 pipeline (ACT works on chunk
ci's M1 while DVE does ci-1's AND; no blocking zigzag).

Buffers ping-pong, no in-place ops, NBUF = n_chunks so no reuse at all:
M1 xt->wt, AND wt->xt, M2 xt->wt, store from wt.

Sems -- every data handoff (same-engine included: engines pipeline, e.g.
an issued DMA can read an earlier op's output before its data lands)
waits on the producer's counting increment:
  load_sem:  +16 per load DMA; DVE waits 16*(ci+1) (one FIFO ring, in-order)
  scal_sem:  +2 per chunk by DVE (inv', d2); ACT M1 waits 2*(ci+1)
  m1_sem:    +1 per ACT M1 slice; DVE AND waits tt*(ci+1)
  and_sem:   +1 per chunk by DVE AND; ACT M2b waits ci+1
  m2a_sem:   +1 per DVE M2 slice;  gp store waits 2*(ci+1)
  m2b_sem:   +1 per ACT M2 slice;  gp store waits 2*(ci+1)
  dve_sem:   DVE-internal RAW fences (reduce1->reduce2->recip->tinies)
  store_sem: +16 per store DMA; sync tail-waits 16*n_chunks (output flushed)
"""

from contextlib import ExitStack

import numpy as np

import concourse.bass as bass
import concourse.mybir as mybir
from concourse.bass_utils import run_bass_kernel_spmd

B, H, T, C = 8, 12, 4096, 64
N_CORES = 8
P = 128          # SBUF partitions
TC = 512         # tokens per chunk (pipeline granularity)

ISQRT2 = 0.7071067811865476
EXP_MASK = 0x7F800000
M2_DVE = 2       # M2 slices 0..M2_DVE-1 on DVE, the rest on ACT

_nc_cache = {}


def _build_nc():
    if "nc" in _nc_cache:
        return _nc_cache["nc"]
    f32 = mybir.dt.float32
    i32 = mybir.dt.int32
    OP = mybir.AluOpType
    AF = mybir.ActivationFunctionType

    nc = bass.Bass()
    x_in = nc.declare_dram_parameter("x", [H, T, C], f32, isOutput=False)
    y_out = nc.declare_dram_parameter("y", [H, T, C], f32, isOutput=True)

    n_chunks = T // TC
    tt = TC // P
    FREE = H * tt * C
    NBUF = n_chunks

    def src_ap(ci):
        return x_in[:, ci * TC : (ci + 1) * TC, :].rearrange(
            "h (p q) c -> p h (q c)", p=P
        )

    def dst_ap(ci):
        return y_out[:, ci * TC : (ci + 1) * TC, :].rearrange(
            "h (p q) c -> p h (q c)", p=P
        )

    with ExitStack() as ctx:
        xt = [
            ctx.enter_context(nc.sbuf_tensor(f"xt{j}", [P, FREE], f32))
            for j in range(NBUF)
        ]
        wt = [
            ctx.enter_context(nc.sbuf_tensor(f"wt{j}", [P, FREE], f32))
            for j in range(NBUF)
        ]
        red = ctx.enter_context(nc.sbuf_tensor("red", [P, H * tt], f32))
        delta = [
            ctx.enter_context(nc.sbuf_tensor(f"delta{j}", [P, tt], f32))
            for j in range(NBUF)
        ]
        inv = [
            ctx.enter_context(nc.sbuf_tensor(f"inv{j}", [P, tt], f32))
            for j in range(NBUF)
        ]
        d2 = [
            ctx.enter_context(nc.sbuf_tensor(f"d2_{j}", [P, tt], f32))
            for j in range(NBUF)
        ]

        load_sem = ctx.enter_context(nc.semaphore("load_sem"))
        store_sem = ctx.enter_context(nc.semaphore("store_sem"))
        scal_sem = ctx.enter_context(nc.semaphore("scal_sem"))
        m1_sem = ctx.enter_context(nc.semaphore("m1_sem"))
        and_sem = ctx.enter_context(nc.semaphore("and_sem"))
        m2a_sem = ctx.enter_context(nc.semaphore("m2a_sem"))
        m2b_sem = ctx.enter_context(nc.semaphore("m2b_sem"))
        dve_sem = ctx.enter_context(nc.semaphore("dve_sem"))

        block = ctx.enter_context(nc.Block())

        def view4(t):
            return t[:].rearrange("p (h q c) -> p h q c", h=H, c=C)

        @block.sync
        def _(sync):
            # loads only; SP HWDGE ring; NBUF == n_chunks so no buffer
            # reuse. Pace issuance to <=3 DMAs in flight: an overfull ring
            # makes dma_start block and throttles the transfers themselves
            # (measured: un-paced loads ran 260 GB/s vs 350 paced).
            for ci in range(n_chunks):
                if ci >= 3:
                    sync.wait_ge(load_sem, 16 * (ci - 2))
                sync.dma_start(out=xt[ci][:], in_=src_ap(ci)).then_inc(
                    load_sem, 16
                )
            # output-flush guarantee before NEFF end
            sync.wait_ge(store_sem, 16 * n_chunks)

        @block.vector
        def _(vector):
            b = 0
            for ci in range(n_chunks):
                xt3 = xt[ci][:].rearrange("p (hq c) -> p hq c", c=C)
                vector.wait_ge(load_sem, 16 * (ci + 1))
                # delta = max over (h, c) in two stages: contiguous X over c,
                # then the tiny 48-element strided reduce over h
                vector.reduce_max(
                    out=red[:], in_=xt3, axis=mybir.AxisListType.X
                ).then_inc(dve_sem, 1)
                vector.wait_ge(dve_sem, b + 1)
                vector.reduce_max(
                    out=delta[ci][:],
                    in_=red[:].rearrange("p (h q) -> p q h", h=H),
                    axis=mybir.AxisListType.X,
                ).then_inc(dve_sem, 1)
                vector.wait_ge(dve_sem, b + 2)
                vector.reciprocal(inv[ci][:], delta[ci][:]).then_inc(dve_sem, 1)
                vector.wait_ge(dve_sem, b + 3)
                # inv' = isqrt2/delta (M1 scale); d2 = 2*delta (M2 scale)
                vector.tensor_scalar_mul(inv[ci][:], inv[ci][:], ISQRT2).then_inc(
                    scal_sem, 1
                )
                vector.tensor_scalar_mul(d2[ci][:], delta[ci][:], 2.0).then_inc(
                    scal_sem, 1
                )
                b += 3
                if ci >= 1:
                    k = ci - 1
                    # AND: p2 = bits(q) & mask, wt -> xt, one flat op
                    vector.wait_ge(m1_sem, tt * ci)
                    vector.tensor_scalar(
                        out=xt[k][:].bitcast(i32),
                        in0=wt[k][:].bitcast(i32),
                        scalar1=EXP_MASK,
                        scalar2=None,
                        op0=OP.bitwise_and,
                    ).then_inc(and_sem, 1)
                    # M2 slices 0..M2_DVE-1: out = p2 * d2, xt -> wt (the
                    # AND above is this stream's own op: in-order + the
                    # consumer-side fence below)
                    vector.wait_ge(and_sem, ci)
                    for s in range(M2_DVE):
                        vector.tensor_scalar_mul(
                            view4(wt[k])[:, :, s, :],
                            view4(xt[k])[:, :, s, :],
                            d2[k][:, s : s + 1],
                        ).then_inc(m2a_sem, 1)
            k = n_chunks - 1
            vector.wait_ge(m1_sem, tt * n_chunks)
            vector.tensor_scalar(
                out=xt[k][:].bitcast(i32),
                in0=wt[k][:].bitcast(i32),
                scalar1=EXP_MASK,
                scalar2=None,
                op0=OP.bitwise_and,
            ).then_inc(and_sem, 1)
            vector.wait_ge(and_sem, n_chunks)
            for s in range(M2_DVE):
                vector.tensor_scalar_mul(
                    view4(wt[k])[:, :, s, :],
                    view4(xt[k])[:, :, s, :],
                    d2[k][:, s : s + 1],
                ).then_inc(m2a_sem, 1)

        @block.scalar
        def _(scalar):
            # M1 (all slices) + M2 slices M2_DVE..tt-1; activation-Copy with
            # per-partition scale AP
            for ci in range(n_chunks):
                scalar.wait_ge(scal_sem, 2 * (ci + 1))
                for s in range(tt):
                    scalar.activation(
                        out=view4(wt[ci])[:, :, s, :],
                        in_=view4(xt[ci])[:, :, s, :],
                        func=AF.Copy,
                        scale=inv[ci][:, s : s + 1],
                    ).then_inc(m1_sem, 1)
                if ci >= 1:
                    k = ci - 1
                    scalar.wait_ge(and_sem, ci)
                    for s in range(M2_DVE, tt):
                        scalar.activation(
                            out=view4(wt[k])[:, :, s, :],
                            in_=view4(xt[k])[:, :, s, :],
                            func=AF.Copy,
                            scale=d2[k][:, s : s + 1],
                        ).then_inc(m2b_sem, 1)
                    # store k; ACT HWDGE ring. BOTH fences are data fences:
                    # in-stream order does NOT imply the DMA reads completed
                    # data (engines pipeline), so self-wait m2b too.
                    scalar.wait_ge(m2a_sem, M2_DVE * ci)
                    scalar.wait_ge(m2b_sem, (tt - M2_DVE) * ci)
                    scalar.dma_start(out=dst_ap(k), in_=wt[k][:]).then_inc(
                        store_sem, 16
                    )
            k = n_chunks - 1
            scalar.wait_ge(and_sem, n_chunks)
            for s in range(M2_DVE, tt):
                scalar.activation(
                    out=view4(wt[k])[:, :, s, :],
                    in_=view4(xt[k])[:, :, s, :],
                    func=AF.Copy,
                    scale=d2[k][:, s : s + 1],
                ).then_inc(m2b_sem, 1)
            scalar.wait_ge(m2a_sem, M2_DVE * n_chunks)
            scalar.wait_ge(m2b_sem, (tt - M2_DVE) * n_chunks)
            scalar.dma_start(out=dst_ap(k), in_=wt[k][:]).then_inc(
                store_sem, 16
            )

    _nc_cache["nc"] = nc
    return nc


def kernel(x: np.ndarray) -> np.ndarray:
    assert x.shape == (B, H, T, C) and x.dtype == np.float32
    nc = _build_nc()
    in_maps = [{"x": np.ascontiguousarray(x[i])} for i in range(N_CORES)]
    res = run_bass_kernel_spmd(nc, in_maps, list(range(N_CORES)))
    out = np.stack([res.results[i]["y"] for i in range(N_CORES)], axis=0)
    return out
